# revision 7
# baseline (speedup 1.0000x reference)
"""Segment-mean (average pooling over sorted segment ids) on 8 TRN2 NeuronCores.

Strategy
--------
segment_ids are sorted, so shard by *segment blocks*: S segments are split
into S/128 blocks of 128 segments; each of the 8 cores owns an equal range
of blocks. On the host, each block's (contiguous) rows are gathered and
padded up to `tau` tiles of 128 rows, giving a fully static instruction
stream shared by all cores (SPMD).

Per 128-row tile the device:
  - builds a one-hot matrix  oh[i, m] = (local_seg_id[row i] == m)  in bf16
    on the VectorEngine (tensor_scalar is_equal against an iota constant),
  - issues ONE matmul  psum[128 segs, 258] += oh.T @ rhs  where
    rhs = [feats_hi | 1 | feats_lo | 0] in bf16.  The hi/lo bf16 split of
    the fp32 features recovers ~fp32 precision while streaming the PE at
    1 cycle/row (fp32 matmul is 4x slower).  The trailing 1-column
    accumulates per-segment counts for free.
Padding rows carry local id -1, so the one-hot zeroes their contribution.

After a block's tau tiles, psum holds [128 segs, 129+129]: the two halves
are added, counts are clamped to >=1, reciprocal'd, and multiplied in
(VectorEngine), then the [128, 128] block mean is DMA'd out.
"""

import os
import sys
from contextlib import ExitStack

import numpy as np

sys.path.insert(0, "/opt/trn_rl_repo")

import ml_dtypes

from concourse import bass, mybir, tile
from concourse.bass_utils import run_bass_kernel_spmd

BF16 = ml_dtypes.bfloat16

N_CORES = 8
P = 128      # rows per tile == partitions == matmul contraction dim
D = 128      # feature dim
BLK = 128    # segments per block == one-hot width == psum partitions
W = 2 * (D + 1)  # rhs free width: [hi(128) | ones(1) | lo(128) | zeros(1)]

# module-level knobs for test.py
TRACE = False
LAST_EXEC_NS = None
CHUNK = 16   # tiles per input DMA (~1MB each)


def _ensure_profile_hook():
    """Register the axon NTFF profile hook if the image's antenv lacks it.

    trn_boot has a ctypes-based hook factory but skips installation when
    `antenv.axon_hooks` is absent; shim the module so trace=True works.
    """
    import types

    try:
        from antenv.axon_hooks import get_axon_ntff_profile_hook  # noqa: F401
        return
    except ImportError:
        pass
    import antenv
    from trn_agent_boot.trn_boot import _ntff_profile_via_ctypes

    mod = types.ModuleType("antenv.axon_hooks")
    _state = {"hook": _ntff_profile_via_ctypes("/opt/axon/libaxon_pjrt.so")}
    mod.set_axon_ntff_profile_hook = lambda h: _state.__setitem__("hook", h)
    mod.get_axon_ntff_profile_hook = lambda: _state["hook"]
    sys.modules["antenv.axon_hooks"] = mod
    antenv.axon_hooks = mod

_prog_cache = {}


def _split_excess_waits(nc, cap_engine=1, cap_nop=1):
    """Walrus enforces per-ISA-struct limits on sync-wait commands (e.g.
    TensorScalarPtr accepts 1). Tile can emit more. Split the excess into
    wait-only NOPs placed immediately before the instruction on the same
    engine — semantically identical (all waits still precede the op)."""
    ctr = [0]
    for f in nc.m.functions:
        for blk in f.blocks:
            insts = blk.instructions
            out = []
            changed = False
            for inst in insts:
                si = inst.sync_info
                waits = list(si.on_wait) if si is not None and si.on_wait else []
                cap = cap_nop if type(inst).__name__ in ("InstNoOp", "InstDrain") else cap_engine
                if len(waits) > cap:
                    excess, keep = waits[:-cap], waits[-cap:]
                    for i in range(0, len(excess), cap_nop):
                        chunk = excess[i : i + cap_nop]
                        ctr[0] += 1
                        nop = mybir.InstNoOp(
                            name=f"W-split-{ctr[0]}",
                            engine=inst.engine,
                            sync_info=mybir.SyncInfo(on_wait=chunk, on_update=[]),
                            ins=[],
                            outs=[],
                            bass_nofuse=True,
                        )
                        out.append(nop)
                    inst.sync_info = mybir.SyncInfo(
                        on_wait=keep, on_update=list(si.on_update) if si.on_update else []
                    )
                    changed = True
                out.append(inst)
            if changed:
                blk.instructions = out
    return nc


def _build_program(tau: int, nblk: int):
    """One SPMD Bass program: nblk blocks x tau tiles per core."""
    nc = bass.Bass()
    T = nblk * tau
    x = nc.declare_dram_parameter("x", [T, P, W], mybir.dt.bfloat16, isOutput=False)
    ids = nc.declare_dram_parameter("ids", [P, T], mybir.dt.float32, isOutput=False)
    iota = nc.declare_dram_parameter("iota", [P, BLK], mybir.dt.bfloat16, isOutput=False)
    out = nc.declare_dram_parameter("out", [nblk, BLK, D], mybir.dt.float32, isOutput=True)

    with tile.TileContext(nc) as tc, ExitStack() as ctx:
        const = ctx.enter_context(tc.tile_pool(name="const", bufs=1))
        xp = ctx.enter_context(tc.tile_pool(name="xp", bufs=3))
        ohp = ctx.enter_context(tc.tile_pool(name="ohp", bufs=8))
        psp = ctx.enter_context(tc.tile_pool(name="psp", bufs=2, space="PSUM"))
        finp = ctx.enter_context(tc.tile_pool(name="finp", bufs=2))

        iota_sb = const.tile([P, BLK], mybir.dt.bfloat16)
        nc.sync.dma_start(iota_sb[:], iota[:])
        ids_sb = const.tile([P, T], mybir.dt.float32)
        nc.sync.dma_start(ids_sb[:], ids[:])
        # warm-up copies: walrus allows only one sync-wait on tensor_scalar,
        # so absorb the two const-DMA semaphores into the DVE's clock first
        warm = const.tile([P, 2], mybir.dt.float32)
        nc.vector.tensor_copy(warm[:, 0:1], ids_sb[:, 0:1])
        nc.vector.tensor_copy(warm[:, 1:2], iota_sb[:, 0:1])

        for b in range(nblk):
            ps = psp.tile([P, W], mybir.dt.float32)
            for k0 in range(0, tau, CHUNK):
                g = min(CHUNK, tau - k0)
                t0 = b * tau + k0
                chunk = xp.tile([P, CHUNK, W], mybir.dt.bfloat16, tag="xchunk")
                nc.sync.dma_start(
                    chunk[:, :g, :], x[t0 : t0 + g].rearrange("g p w -> p g w")
                )
                for kk in range(g):
                    k = k0 + kk
                    t = t0 + kk
                    oh = ohp.tile([P, BLK], mybir.dt.bfloat16, tag="oh")
                    nc.vector.tensor_scalar(
                        oh[:],
                        iota_sb[:],
                        ids_sb[:, t : t + 1],
                        None,
                        mybir.AluOpType.is_equal,
                    )
                    nc.tensor.matmul(
                        ps[:],
                        oh[:],
                        chunk[:, kk, :],
                        start=(k == 0),
                        stop=(k == tau - 1),
                    )
            # finalize block: mean = (hi_sum + lo_sum) / max(count, 1)
            evac = finp.tile([P, W], mybir.dt.float32, tag="evac")
            nc.vector.tensor_copy(evac[:], ps[:])
            sums = finp.tile([P, D + 1], mybir.dt.float32, tag="sums")
            nc.vector.tensor_add(sums[:], evac[:, 0 : D + 1], evac[:, D + 1 : W])
            cnt = finp.tile([P, 1], mybir.dt.float32, tag="cnt")
            nc.vector.tensor_scalar_max(cnt[:], sums[:, D : D + 1], 1.0)
            rcp = finp.tile([P, 1], mybir.dt.float32, tag="rcp")
            nc.vector.reciprocal(rcp[:], cnt[:])
            osb = finp.tile([P, D], mybir.dt.float32, tag="osb")
            nc.vector.tensor_scalar(
                osb[:], sums[:, 0:D], rcp[:], None, mybir.AluOpType.mult
            )
            nc.sync.dma_start(out[b], osb[:])
    return _split_excess_waits(nc)


def kernel(feats, segment_ids, num_segments):
    global LAST_EXEC_NS
    feats = np.asarray(feats, dtype=np.float32)
    segment_ids = np.asarray(segment_ids, dtype=np.int32)
    S = int(num_segments)
    N = feats.shape[0]
    assert feats.shape[1] == D
    assert S % (N_CORES * BLK) == 0, f"num_segments={S} must divide into 8x128 blocks"
    seg_per_core = S // N_CORES
    nblk = seg_per_core // BLK
    nblocks_total = S // BLK

    # rows of each 128-segment block (ids are sorted)
    bounds = np.searchsorted(segment_ids, np.arange(0, S + 1, BLK))
    rows_per_block = np.diff(bounds)
    tau = max(1, int(-(-int(rows_per_block.max()) // P)))
    T = nblk * tau

    iota_np = np.ascontiguousarray(
        np.broadcast_to(np.arange(BLK, dtype=np.float32), (P, BLK))
    ).astype(BF16)

    in_maps = []
    for c in range(N_CORES):
        idx = np.zeros((nblk, tau * P), dtype=np.int64)
        sid = np.full((nblk, tau * P), -1.0, dtype=np.float32)
        for bi in range(nblk):
            gb = c * nblk + bi
            r0, r1 = int(bounds[gb]), int(bounds[gb + 1])
            n = r1 - r0
            assert n <= tau * P
            idx[bi, :n] = np.arange(r0, r1)
            sid[bi, :n] = segment_ids[r0:r1].astype(np.float32) - gb * BLK
        f = feats[idx.reshape(-1)]  # [T*P, D]; pad rows point at row 0, masked by one-hot
        hi = f.astype(BF16)
        lo = (f - hi.astype(np.float32)).astype(BF16)
        Xc = np.empty((T, P, W), dtype=BF16)
        Xc[:, :, 0:D] = hi.reshape(T, P, D)
        Xc[:, :, D] = 1.0
        Xc[:, :, D + 1 : W - 1] = lo.reshape(T, P, D)
        Xc[:, :, W - 1] = 0.0
        idsc = np.ascontiguousarray(sid.reshape(T, P).T)  # [P, T] f32
        in_maps.append({"x": Xc, "ids": idsc, "iota": iota_np})

    key = (tau, nblk)
    if key not in _prog_cache:
        _prog_cache[key] = _build_program(tau, nblk)
    nc = _prog_cache[key]

    if TRACE:
        _ensure_profile_hook()
    res = run_bass_kernel_spmd(
        nc, in_maps, core_ids=list(range(N_CORES)), trace=TRACE
    )
    LAST_EXEC_NS = res.exec_time_ns
    outs = [
        np.asarray(res.results[c]["out"]).reshape(seg_per_core, D)
        for c in range(N_CORES)
    ]
    return np.concatenate(outs, axis=0).astype(np.float32)


# revision 13
# speedup vs baseline: 1.1064x; 1.1064x over previous
"""Segment-mean (average pooling over sorted segment ids) on 8 TRN2 NeuronCores.

Strategy
--------
segment_ids are sorted, so shard by *segment blocks*: S segments are split
into S/128 blocks of 128 segments; each of the 8 cores owns an equal range
of blocks. On the host, each block's (contiguous) rows are gathered and
padded up to `tau` tiles of 128 rows, giving a fully static instruction
stream shared by all cores (SPMD).

Per 128-row tile the device:
  - builds a one-hot matrix  oh[i, m] = (windowed_seg_id[row i] == m)  in
    bf16 on the VectorEngine (tensor_scalar is_equal against an iota
    constant),
  - issues ONE matmul  psum[w_k : w_k+64, 258] += oh.T @ rhs  where
    rhs = [feats_hi | 1 | feats_lo | 0] in bf16.  The hi/lo bf16 split of
    the fp32 features recovers ~fp32 precision while streaming the PE at
    1 cycle/row (fp32 matmul is 4x slower).  The trailing 1-column
    accumulates per-segment counts for free.

The matmul's stationary (one-hot) is only 64 segments wide: within a
128-segment block, the segments seen by tile k sit in a narrow,
data-predictable band, so a 32-aligned 64-wide window w(k) — a pure
function of k, identical across blocks and cores, verified on the host —
contains them. This halves the dominant LDWEIGHTS cost. Tile k=0 uses the
full 128-wide one-hot with start=True to initialize the whole PSUM
accumulator (has_written semantics), later tiles accumulate into their
window. Padding rows carry id -1, so the one-hot zeroes their contribution.

After a block's tau tiles, psum holds [128 segs, 129+129]: the two halves
are added, counts are clamped to >=1, reciprocal'd, and multiplied in
(VectorEngine), then the [128, 128] block mean is DMA'd out.

Host-side input layout is [128 partitions, T tiles, 258], so every
partition streams long contiguous runs (multi-KB DMA descriptors).
"""

import os
import sys
from contextlib import ExitStack

import numpy as np

sys.path.insert(0, "/opt/trn_rl_repo")

import ml_dtypes

from concourse import bass, mybir, tile
from concourse.bass_utils import run_bass_kernel_spmd

BF16 = ml_dtypes.bfloat16

N_CORES = 8
P = 128      # rows per tile == partitions == matmul contraction dim
D = 128      # feature dim
BLK = 128    # segments per block == psum partitions
WIN = 64     # one-hot window width (stationary columns) for k > 0
W = 2 * (D + 1)  # rhs free width: [hi(128) | ones(1) | lo(128) | zeros(1)]

# module-level knobs for test.py
TRACE = False
LAST_EXEC_NS = None
CHUNK = 32   # tiles per input DMA (~2MB each)

_prog_cache = {}


def _ensure_profile_hook():
    """Register the axon NTFF profile hook if the image's antenv lacks it.

    trn_boot has a ctypes-based hook factory but skips installation when
    `antenv.axon_hooks` is absent; shim the module so trace=True works.
    """
    import types

    try:
        from antenv.axon_hooks import get_axon_ntff_profile_hook  # noqa: F401
        return
    except ImportError:
        pass
    import antenv
    from trn_agent_boot.trn_boot import _ntff_profile_via_ctypes

    mod = types.ModuleType("antenv.axon_hooks")
    _state = {"hook": _ntff_profile_via_ctypes("/opt/axon/libaxon_pjrt.so")}
    mod.set_axon_ntff_profile_hook = lambda h: _state.__setitem__("hook", h)
    mod.get_axon_ntff_profile_hook = lambda: _state["hook"]
    sys.modules["antenv.axon_hooks"] = mod
    antenv.axon_hooks = mod


def _split_excess_waits(nc, cap=1):
    """Walrus enforces a limit of one sync-wait command per instruction.
    Tile can emit more. Split the excess into wait-only NOPs placed
    immediately before the instruction on the same engine — semantically
    identical (all waits still precede the op)."""
    ctr = [0]
    for f in nc.m.functions:
        for blk in f.blocks:
            insts = blk.instructions
            out = []
            changed = False
            for inst in insts:
                si = inst.sync_info
                waits = list(si.on_wait) if si is not None and si.on_wait else []
                if len(waits) > cap:
                    excess, keep = waits[:-cap], waits[-cap:]
                    for i in range(0, len(excess), cap):
                        chunk = excess[i : i + cap]
                        ctr[0] += 1
                        nop = mybir.InstNoOp(
                            name=f"W-split-{ctr[0]}",
                            engine=inst.engine,
                            sync_info=mybir.SyncInfo(on_wait=chunk, on_update=[]),
                            ins=[],
                            outs=[],
                            bass_nofuse=True,
                        )
                        out.append(nop)
                    inst.sync_info = mybir.SyncInfo(
                        on_wait=keep, on_update=list(si.on_update) if si.on_update else []
                    )
                    changed = True
                out.append(inst)
            if changed:
                blk.instructions = out
    return nc


def _build_program(tau: int, nblk: int, plan: tuple):
    """One SPMD Bass program: nblk blocks x tau tiles per core.

    plan[k] = (psum-partition base, width) of tile k's one-hot window
    (plan[0] == (0, 128): tile 0 initializes the whole accumulator)."""
    nc = bass.Bass()
    T = nblk * tau
    x = nc.declare_dram_parameter("x", [P, T, W], mybir.dt.bfloat16, isOutput=False)
    ids = nc.declare_dram_parameter("ids", [P, T], mybir.dt.float32, isOutput=False)
    iota = nc.declare_dram_parameter("iota", [P, BLK], mybir.dt.bfloat16, isOutput=False)
    out = nc.declare_dram_parameter("out", [nblk, BLK, D], mybir.dt.float32, isOutput=True)

    with tile.TileContext(nc) as tc, ExitStack() as ctx:
        const = ctx.enter_context(tc.tile_pool(name="const", bufs=1))
        xp = ctx.enter_context(tc.tile_pool(name="xp", bufs=3))
        ohp = ctx.enter_context(tc.tile_pool(name="ohp", bufs=8))
        psp = ctx.enter_context(tc.tile_pool(name="psp", bufs=2, space="PSUM"))
        finp = ctx.enter_context(tc.tile_pool(name="finp", bufs=2))

        iota_sb = const.tile([P, BLK], mybir.dt.bfloat16)
        nc.sync.dma_start(iota_sb[:], iota[:])
        ids_sb = const.tile([P, T], mybir.dt.float32)
        nc.sync.dma_start(ids_sb[:], ids[:])
        # warm-up copies: absorb the two const-DMA semaphores into the DVE's
        # clock so the first one-hot op carries at most one sync wait
        warm = const.tile([P, 2], mybir.dt.float32)
        nc.vector.tensor_copy(warm[:, 0:1], ids_sb[:, 0:1])
        nc.vector.tensor_copy(warm[:, 1:2], iota_sb[:, 0:1])

        for b in range(nblk):
            ps = psp.tile([P, W], mybir.dt.float32)
            for k0 in range(0, tau, CHUNK):
                g = min(CHUNK, tau - k0)
                t0 = b * tau + k0
                chunk = xp.tile([P, CHUNK, W], mybir.dt.bfloat16, tag="xchunk")
                nc.sync.dma_start(chunk[:, :g, :], x[:, t0 : t0 + g, :])
                for kk in range(g):
                    k = k0 + kk
                    t = t0 + kk
                    wbase, width = plan[k]
                    oh = ohp.tile([P, BLK], mybir.dt.bfloat16, tag="oh")
                    nc.vector.tensor_scalar(
                        oh[:, :width],
                        iota_sb[:, :width],
                        ids_sb[:, t : t + 1],
                        None,
                        mybir.AluOpType.is_equal,
                    )
                    nc.tensor.matmul(
                        ps[wbase : wbase + width, :],
                        oh[:, :width],
                        chunk[:, kk, :],
                        tile_position=(0, wbase),
                        start=(k == 0),
                        stop=(k == tau - 1),
                        skip_group_check=True,
                    )
            # finalize block: mean = (hi_sum + lo_sum) / max(count, 1)
            evac = finp.tile([P, W], mybir.dt.float32, tag="evac")
            nc.vector.tensor_copy(evac[:], ps[:])
            sums = finp.tile([P, D + 1], mybir.dt.float32, tag="sums")
            nc.vector.tensor_add(sums[:], evac[:, 0 : D + 1], evac[:, D + 1 : W])
            cnt = finp.tile([P, 1], mybir.dt.float32, tag="cnt")
            nc.vector.tensor_scalar_max(cnt[:], sums[:, D : D + 1], 1.0)
            rcp = finp.tile([P, 1], mybir.dt.float32, tag="rcp")
            nc.vector.reciprocal(rcp[:], cnt[:])
            osb = finp.tile([P, D], mybir.dt.float32, tag="osb")
            nc.vector.tensor_scalar(
                osb[:], sums[:, 0:D], rcp[:], None, mybir.AluOpType.mult
            )
            nc.sync.dma_start(out[b], osb[:])
    return _split_excess_waits(nc)


def _plan_windows(segment_ids, bounds, nblocks_total, tau):
    """Choose the one-hot window (base w, width) per tile index k, valid for
    every block instance. Matmul output-partition alignment requires width-32
    windows to start at multiples of 32, width-64 at {0, 64}, width-128 at 0.
    Tile 0 always gets (0, 128) — it initializes the whole accumulator."""
    lo = np.full(tau, BLK, dtype=np.int64)
    hi = np.full(tau, -1, dtype=np.int64)
    for gb in range(nblocks_total):
        r0, r1 = int(bounds[gb]), int(bounds[gb + 1])
        n = r1 - r0
        if n == 0:
            continue
        sid = segment_ids[r0:r1]
        base = gb * BLK
        kmax = -(-n // P)
        for k in range(kmax):
            a = sid[k * P] - base
            bnd = sid[min((k + 1) * P, n) - 1] - base
            if a < lo[k]:
                lo[k] = a
            if bnd > hi[k]:
                hi[k] = bnd
    plan = []
    for k in range(tau):
        if k == 0 or hi[k] < 0:
            plan.append((0, BLK))
            continue
        chosen = None
        for width in (32, 64, 128):
            for w in range(0, BLK - width + 1, width):
                if w <= lo[k] and hi[k] < w + width:
                    chosen = (w, width)
                    break
            if chosen:
                break
        assert chosen is not None  # width=128, w=0 always covers
        plan.append(chosen)
    return tuple(plan)


def kernel(feats, segment_ids, num_segments):
    global LAST_EXEC_NS
    feats = np.asarray(feats, dtype=np.float32)
    segment_ids = np.asarray(segment_ids, dtype=np.int32)
    S = int(num_segments)
    N = feats.shape[0]
    assert feats.shape[1] == D
    assert S % (N_CORES * BLK) == 0, f"num_segments={S} must divide into 8x128 blocks"
    seg_per_core = S // N_CORES
    nblk = seg_per_core // BLK
    nblocks_total = S // BLK

    # rows of each 128-segment block (ids are sorted)
    bounds = np.searchsorted(segment_ids, np.arange(0, S + 1, BLK))
    rows_per_block = np.diff(bounds)
    tau = max(1, int(-(-int(rows_per_block.max()) // P)))
    T = nblk * tau

    plan = _plan_windows(segment_ids, bounds, nblocks_total, tau)

    iota_np = np.ascontiguousarray(
        np.broadcast_to(np.arange(BLK, dtype=np.float32), (P, BLK))
    ).astype(BF16)

    # per-row window base: rows of tile k get offset gb*BLK + plan[k][0]
    wk_arr = np.asarray([p_[0] for p_ in plan], dtype=np.int64)

    in_maps = []
    for c in range(N_CORES):
        idx = np.zeros((nblk, tau, P), dtype=np.int64)
        sid = np.full((nblk, tau, P), -1.0, dtype=np.float32)
        for bi in range(nblk):
            gb = c * nblk + bi
            r0, r1 = int(bounds[gb]), int(bounds[gb + 1])
            n = r1 - r0
            assert n <= tau * P
            flat_idx = idx[bi].reshape(-1)
            flat_sid = sid[bi].reshape(-1)
            flat_idx[:n] = np.arange(r0, r1)
            local = segment_ids[r0:r1].astype(np.float32) - gb * BLK
            # subtract per-tile window base
            koff = np.repeat(wk_arr, P)[:n].astype(np.float32)
            flat_sid[:n] = local - koff
        idxT = idx.reshape(T, P).T  # [P, T]
        f = feats[idxT.reshape(-1)]  # [P*T, D]; pad rows point at row 0, masked
        hi = f.astype(BF16)
        lo = (f - hi.astype(np.float32)).astype(BF16)
        Xc = np.empty((P, T, W), dtype=BF16)
        Xc[:, :, 0:D] = hi.reshape(P, T, D)
        Xc[:, :, D] = 1.0
        Xc[:, :, D + 1 : W - 1] = lo.reshape(P, T, D)
        Xc[:, :, W - 1] = 0.0
        idsc = np.ascontiguousarray(sid.reshape(T, P).T)  # [P, T] f32
        in_maps.append({"x": Xc, "ids": idsc, "iota": iota_np})

    key = (tau, nblk, plan)
    if key not in _prog_cache:
        _prog_cache[key] = _build_program(tau, nblk, plan)
    nc = _prog_cache[key]

    if TRACE:
        _ensure_profile_hook()
    res = run_bass_kernel_spmd(
        nc, in_maps, core_ids=list(range(N_CORES)), trace=TRACE
    )
    LAST_EXEC_NS = res.exec_time_ns
    outs = [
        np.asarray(res.results[c]["out"]).reshape(seg_per_core, D)
        for c in range(N_CORES)
    ]
    return np.concatenate(outs, axis=0).astype(np.float32)
